# revision 2
# baseline (speedup 1.0000x reference)
"""Block-sparse position-wise FFN on Trainium2 (Bass/Tile), 8-core data-parallel.

v2 strategy (vs baseline):
  - bf16 stationary+moving operands (fp32 PSUM): LDWEIGHTS uses FWL (2x)
    and is fully hidden under the N-column stream; per-MM ~163ns vs
    ~175-190ns for fp32r whose 64KB weight load wasn't hidden.
  - Joint clustering optimization over THREE permutations (fc1 d-windows,
    shared f-groups, fc2 d-tiles) with randomized-restart greedy + swap
    polish: 268 matmuls/segment vs 280 (dense 288). The natural d-order
    is pessimal for this mask (strip-empty 0.069 vs 0.168 randomized).
  - 512-column segments ([512]*8 + [260]*2 per core) to amortize per-MM
    issue overhead; 512 fp32 = exactly one PSUM bank.
  - Tokens sharded 8 ways (4616/core); weights replicated; no collectives.
"""

import sys
import types

import numpy as np

try:
    import antenv.axon_hooks  # noqa: F401
except ImportError:
    import antenv

    _hooks = types.ModuleType("antenv.axon_hooks")
    _hooks._hook = None
    _hooks.set_axon_ntff_profile_hook = (
        lambda h: setattr(_hooks, "_hook", h))
    _hooks.get_axon_ntff_profile_hook = lambda: _hooks._hook
    sys.modules["antenv.axon_hooks"] = _hooks
    antenv.axon_hooks = _hooks

import ml_dtypes
import concourse.bass as bass
import concourse.bacc as bacc
import concourse.mybir as mybir
from concourse import tile
from concourse.bass_utils import run_bass_kernel_spmd

B, S, DIM, FF, BLK = 64, 577, 768, 3072, 8
NCORES = 8
TOK = B * S                # 36928
T = TOK // NCORES          # 4616 tokens per core
P = 128
KD = DIM // P              # 6 d-tiles
KF = FF // P               # 24 f-tiles
SEGW = 392                 # max segment width
F32 = mybir.dt.float32
BF16 = mybir.dt.bfloat16
GELU = mybir.ActivationFunctionType.Gelu
IDENT = mybir.ActivationFunctionType.Identity

SEGS = [384] * 11 + [392]               # sums to 4616
assert sum(SEGS) == T


def _body(tc, x_d, w1_d, b1_d, w2_d, b2_d, o_d, K1, K2):
    nc = tc.nc
    with (
        tc.tile_pool(name="const", bufs=1) as constp,
        tc.tile_pool(name="wpool", bufs=1) as wp,
        tc.tile_pool(name="xt", bufs=2) as xtp,
        tc.tile_pool(name="ht", bufs=25) as htp,
        tc.tile_pool(name="onat", bufs=3) as onatp,
        tc.tile_pool(name="ps1", bufs=3, space=bass.MemorySpace.PSUM) as ps1p,
        tc.tile_pool(name="ps2", bufs=3, space=bass.MemorySpace.PSUM) as ps2p,
    ):
        b1_s = constp.tile([P, KF], F32)
        b2_s = constp.tile([P, KD], F32)

        # fc1 weights: w1_s[k] = W1mT[k*128:(k+1)*128, :]  ([128 d, 3072 f])
        # loaded in 8 column chunks so fc1 can start after the first chunk.
        w1_s = [wp.tile([P, FF], BF16, tag=f"w1_{k}", name=f"w1_{k}")
                for k in range(KD)]
        W1CHUNK = FF // 8
        for cc in range(8):
            for k in range(KD):
                nc.sync.dma_start(
                    out=w1_s[k][:, cc * W1CHUNK:(cc + 1) * W1CHUNK],
                    in_=w1_d[k * P:(k + 1) * P,
                             cc * W1CHUNK:(cc + 1) * W1CHUNK],
                )
            if cc == 0:
                nc.sync.dma_start(out=b1_s[:], in_=b1_d)
                nc.sync.dma_start(out=b2_s[:], in_=b2_d)
        # fc2 weights: w2_s[k] = W2mT[k*128:(k+1)*128, :]  ([128 f, 768 d])
        w2_s = []
        for k in range(KF):
            w = wp.tile([P, DIM], BF16, tag=f"w2_{k}")
            nc.sync.dma_start(out=w[:], in_=w2_d[k * P:(k + 1) * P, :])
            w2_s.append(w)

        s0 = 0
        for w in SEGS:
            # x arrives pre-transposed+permuted from host: [128 d, w tokens]
            xts = []
            for k in range(KD):
                xt = xtp.tile([P, SEGW], BF16, tag=f"xt{k}", name=f"xt{k}")
                nc.gpsimd.dma_start(
                    out=xt[:, 0:w], in_=x_d[k * P:(k + 1) * P, s0:s0 + w]
                )
                xts.append(xt)

            # --- fc1: hT[m] = gelu(W1mT[:,m].T @ xT + b1[m]) ---
            hts = []
            for m in range(KF):
                ps = ps1p.tile([P, SEGW], F32, tag="ps1", name="ps")
                ks = K1[m]
                for j, k in enumerate(ks):
                    nc.tensor.matmul(
                        ps[:, 0:w],
                        w1_s[k][:, m * P:(m + 1) * P],
                        xts[k][:, 0:w],
                        start=(j == 0), stop=(j == len(ks) - 1),
                    )
                ht = htp.tile([P, SEGW], BF16, tag="ht", name="ht")
                nc.scalar.activation(
                    ht[:, 0:w], ps[:, 0:w], GELU, bias=b1_s[:, m:m + 1]
                )
                hts.append(ht)

            # --- fc2: outT[g] = W2mT[:,g].T @ hT + b2[g] ---
            for g in range(KD):
                ps = ps2p.tile([P, SEGW], F32, tag="ps2", name="ps")
                ks2 = K2[g]
                for j, k in enumerate(ks2):
                    nc.tensor.matmul(
                        ps[:, 0:w],
                        w2_s[k][:, g * P:(g + 1) * P],
                        hts[k][:, 0:w],
                        start=(j == 0), stop=(j == len(ks2) - 1),
                    )
                on = onatp.tile([P, SEGW], F32, tag="on", name="on")
                nc.scalar.activation(
                    on[:, 0:w], ps[:, 0:w], IDENT, bias=b2_s[:, g:g + 1]
                )
                nc.sync.dma_start(
                    out=o_d[g * P:(g + 1) * P, s0:s0 + w], in_=on[:, 0:w]
                )
            s0 += w


def build_program(K1, K2, t_tokens=T):
    nc = bacc.Bacc("TRN2", target_bir_lowering=False, debug=False,
                   num_devices=NCORES)
    x_d = nc.dram_tensor("xt", [DIM, t_tokens], BF16,
                         kind="ExternalInput").ap()
    w1_d = nc.dram_tensor("w1t", [DIM, FF], BF16, kind="ExternalInput").ap()
    b1_d = nc.dram_tensor("b1", [P, KF], F32, kind="ExternalInput").ap()
    w2_d = nc.dram_tensor("w2t", [FF, DIM], BF16, kind="ExternalInput").ap()
    b2_d = nc.dram_tensor("b2", [P, KD], F32, kind="ExternalInput").ap()
    o_d = nc.dram_tensor("out", [DIM, t_tokens], F32,
                         kind="ExternalOutput").ap()
    with tile.TileContext(nc) as tc:
        _body(tc, x_d, w1_d, b1_d, w2_d, b2_d, o_d, K1, K2)
    nc.compile()
    return nc


# ---------------- clustering optimizer (host, compile-time) ----------------

def _cost1(m1, pf, pd1):
    return m1[np.ix_(pf, pd1)].reshape(24, 16, 6, 16).any(axis=(1, 3))


def _cost2(m2, pd2, pf):
    return m2[np.ix_(pd2, pf)].reshape(6, 16, 24, 16).any(axis=(1, 3))


def _greedy_cluster(sup, tsz):
    nb, nwin = sup.shape
    unassigned = list(range(nb))
    tiles = []
    while True:
        best_w, best_av = None, None
        for w in range(nwin):
            av = [f for f in unassigned if not sup[f, w]]
            if len(av) >= tsz and (best_av is None or len(av) > len(best_av)):
                best_w, best_av = w, av
        if best_w is None:
            break
        best_av.sort(key=lambda f: int((~sup[f]).sum()))
        take = best_av[:tsz]
        tiles.append(take)
        for f in take:
            unassigned.remove(f)
    while unassigned:
        tiles.append(unassigned[:tsz])
        unassigned = unassigned[tsz:]
    return np.array([f for t in tiles for f in t])


def _polish(m1, m2, pf, pd1, pd2, iters, rng):
    c1 = _cost1(m1, pf, pd1)
    c2 = _cost2(m2, pd2, pf)
    cur = int(c1.sum() + c2.sum())
    for _ in range(iters):
        u = rng.random()
        if u < 0.5:
            i, j = rng.integers(384, size=2)
            if i // 16 == j // 16:
                continue
            pf[[i, j]] = pf[[j, i]]
            n1 = _cost1(m1, pf, pd1)
            n2 = _cost2(m2, pd2, pf)
            nn = int(n1.sum() + n2.sum())
            if nn <= cur:
                cur, c1, c2 = nn, n1, n2
            else:
                pf[[i, j]] = pf[[j, i]]
        elif u < 0.75:
            i, j = rng.integers(96, size=2)
            if i // 16 == j // 16:
                continue
            pd1[[i, j]] = pd1[[j, i]]
            n1 = _cost1(m1, pf, pd1)
            nn = int(n1.sum() + c2.sum())
            if nn <= cur:
                cur, c1 = nn, n1
            else:
                pd1[[i, j]] = pd1[[j, i]]
        else:
            i, j = rng.integers(96, size=2)
            if i // 16 == j // 16:
                continue
            pd2[[i, j]] = pd2[[j, i]]
            n2 = _cost2(m2, pd2, pf)
            nn = int(c1.sum() + n2.sum())
            if nn <= cur:
                cur, c2 = nn, n2
            else:
                pd2[[i, j]] = pd2[[j, i]]
    return pf, pd1, pd2, cur


def _plan(mask1, mask2, restarts=4, polish_iters=30000):
    m1 = np.asarray(mask1, dtype=bool)
    m2 = np.asarray(mask2, dtype=bool)
    rng = np.random.default_rng(0)
    best = None
    for _ in range(restarts):
        pd1 = rng.permutation(96)
        sup1 = m1[:, pd1].reshape(384, 6, 16).any(2)
        pf = _greedy_cluster(sup1, 16)
        sup2 = m2[:, pf].reshape(96, 24, 16).any(2)
        pd2 = _greedy_cluster(sup2, 16)
        pf, pd1, pd2, cur = _polish(m1, m2, pf.copy(), pd1.copy(),
                                    pd2.copy(), polish_iters, rng)
        if best is None or cur < best[0]:
            best = (cur, pf.copy(), pd1.copy(), pd2.copy())
    _, pf, pd1, pd2 = best
    t1 = _cost1(m1, pf, pd1)
    t2 = _cost2(m2, pd2, pf)
    K1 = [[int(k) for k in range(6) if t1[m, k]] or [0] for m in range(24)]
    K2 = [[int(k) for k in range(24) if t2[g, k]] or [0] for g in range(6)]
    return pf, pd1, pd2, K1, K2


def _rows(blocks):
    return (np.asarray(blocks)[:, None] * BLK + np.arange(BLK)[None, :]
            ).ravel()


def host_prep(x, W1, b1, W2, b2, mask1, mask2, pf, pd1, pd2):
    frows = _rows(pf)          # 3072
    d1rows = _rows(pd1)        # 768
    d2rows = _rows(pd2)        # 768
    m1 = np.repeat(np.repeat(np.asarray(mask1, dtype=bool), BLK, 0), BLK, 1)
    m2 = np.repeat(np.repeat(np.asarray(mask2, dtype=bool), BLK, 0), BLK, 1)
    w1m = np.asarray(W1, np.float32) * m1.astype(np.float32)
    w2m = np.asarray(W2, np.float32) * m2.astype(np.float32)
    xt = np.ascontiguousarray(
        np.asarray(x, np.float32).reshape(TOK, DIM).T[d1rows]
    ).astype(ml_dtypes.bfloat16)                                # [DIM, TOK]
    w1t = np.ascontiguousarray(
        w1m[np.ix_(frows, d1rows)].T).astype(ml_dtypes.bfloat16)  # [768,3072]
    w2t = np.ascontiguousarray(
        w2m[np.ix_(d2rows, frows)].T).astype(ml_dtypes.bfloat16)  # [3072,768]
    b1h = np.ascontiguousarray(
        np.asarray(b1, np.float32)[frows].reshape(KF, P).T)       # [P, KF]
    b2h = np.ascontiguousarray(
        np.asarray(b2, np.float32)[d2rows].reshape(KD, P).T)      # [P, KD]
    return xt, w1t, b1h, w2t, b2h, d2rows


_PROGRAM = None
_PROGRAM_KEY = None


def _get_program(mask1, mask2):
    global _PROGRAM, _PROGRAM_KEY
    key = (np.asarray(mask1).tobytes(), np.asarray(mask2).tobytes())
    if _PROGRAM is None or _PROGRAM_KEY != key:
        pf, pd1, pd2, K1, K2 = _plan(mask1, mask2)
        _PROGRAM = (build_program(K1, K2, T), pf, pd1, pd2)
        _PROGRAM_KEY = key
    return _PROGRAM


def kernel(x, W1, b1, W2, b2, mask1, mask2, **run_kwargs):
    nc, pf, pd1, pd2 = _get_program(mask1, mask2)
    xt, w1t, b1h, w2t, b2h, d2rows = host_prep(
        x, W1, b1, W2, b2, mask1, mask2, pf, pd1, pd2)
    in_maps = [
        {"xt": np.ascontiguousarray(xt[:, c * T:(c + 1) * T]),
         "w1t": w1t, "b1": b1h, "w2t": w2t, "b2": b2h}
        for c in range(NCORES)
    ]
    res = run_bass_kernel_spmd(nc, in_maps, list(range(NCORES)), **run_kwargs)
    out = np.concatenate(
        [res.results[c]["out"] for c in range(NCORES)], axis=1)  # [DIMp, TOK]
    inv = np.empty(DIM, np.int64)
    inv[d2rows] = np.arange(DIM)
    out = out[inv]
    out = np.ascontiguousarray(out.T).reshape(B, S, DIM).astype(np.float32)
    if run_kwargs:
        kernel.last_results = res
    return out
